# revision 1
# baseline (speedup 1.0000x reference)
"""Trainium2 Bass kernel for BinaryMaskPredictor (ragged anchors).

Data-parallel over the 256 anchors: 32 anchors per NeuronCore on 8 cores.
feature_map / seg / conv weights are replicated; per-core anchor coords and
target classes are sharded.  Each core computes sum over its anchors of
sum_px BCE(logits, tgt); the host sums the 8 partial scalars and normalizes.

Per-anchor pipeline on device (matmuls in float32r at 1 cyc/row; fp32r
matmuls require base partition 0 destinations, so everything is per-anchor
at partition base 0):
  1. DMA the 32x32x128 feature crop (dynamic y0/x0 via SP registers) into a
     zero-padded [128, 34*34] SBUF tile.
  2. conv1 (128->256ch, 3x3 SAME) as 9-tap shift-and-matmul, K=128(ci),
     M=128(co half), N=512; ACT applies bias+relu into a padded h tile.
  3. conv2 stage A: per-tap partials Z[m, q] = sum_ci h[ci,q]*W2[ci,m]
     (K=128, M=9) accumulated over the two ci halves, copied to SBUF.
  4. conv2 stage B: logits[q] = sum_m Z[m, q+shift_m] via 9 accumulating
     K=9, M=1 matmuls against unit columns of a 9x9 identity.
  5. BCE on partition 0: relu(x) - x*t + ln(1+exp(-|x|)) with x = L + b2,
     fused row-sums on ACT/DVE; tgt = (seg crop == tgt_class) compare.
  6. Per-anchor-slot accumulator R4[4,1] summed across groups, DMA'd out;
     the host sums the 8x4 partials and normalizes.

Performance state (TimelineSim cost model; NTFF unavailable in container):
  426 us/core; PE busy ~330 us.  Gaps: 3x18.8 us back-edge stalls (staggered
  For_i allows only 1-stage skew, so the BCE tail gates the next body),
  ~13 us prologue, ~14 us kernel drain.  Tried and rejected (all neutral or
  worse in the cost model): 16-anchor bodies, feat-DMA split onto Act HWDGE,
  PSUM pool rebalances, explicit stage_boundary placements.  hint_engines=
  (PE,) is kept: the ~372-instruction PE body exceeds one IRAM block, so the
  back-edge branch would I$-miss (~3-4 us/edge on silicon, unmodeled in sim).
  Next real lever: share conv1 across overlapping crops (~2.2x less conv1
  work) via y-sorted anchor assignment + border fixups.
"""

import numpy as np
from contextlib import ExitStack

C = 128
HF = WF = 320
IMG = 1280
NANCH = 256
CROP = 32
PAD = CROP + 2          # 34
NPAD = PAD * PAD        # 1156
NPX = CROP * CROP       # 1024
WPAD = CROP + 2         # 34 (x-padded row pitch)
NXP = CROP * WPAD       # 1088
NCORES = 8
APC = NANCH // NCORES   # 32 anchors per core
GRP = 4                 # anchors per stage-B stack (PSUM partition blocks)
NSUB = 2                # sub-groups unrolled per loop body
NBODY = APC // (GRP * NSUB)  # 4 loop iterations per core
NUM_BASE = 64

_cache = {}
last_exec_time_ns = None
last_results = None


def _build_program():
    import concourse.bass as bass
    import concourse.tile as tile
    import concourse.mybir as mybir
    from concourse import bacc
    from concourse.bass import ds

    f32 = mybir.dt.float32
    f32r = mybir.dt.float32r
    i32 = mybir.dt.int32
    AF = mybir.ActivationFunctionType
    OP = mybir.AluOpType

    nc = bacc.Bacc("TRN2", target_bir_lowering=False, debug=False,
                   num_devices=NCORES)

    feat = nc.declare_dram_parameter("feat", [C, HF, WF], f32r, isOutput=False)
    seg = nc.declare_dram_parameter("seg", [IMG, IMG], i32, isOutput=False)
    coords = nc.declare_dram_parameter("coords", [1, 2 * APC], i32, isOutput=False)
    clsv = nc.declare_dram_parameter("clsv", [1, APC], f32, isOutput=False)
    w1t = nc.declare_dram_parameter("w1t", [9, C, 256], f32r, isOutput=False)
    w2t = nc.declare_dram_parameter("w2t", [C, 18], f32r, isOutput=False)
    b1t = nc.declare_dram_parameter("b1t", [C, 2], f32, isOutput=False)
    b2t = nc.declare_dram_parameter("b2t", [C, 1], f32, isOutput=False)
    e36 = nc.declare_dram_parameter("e36", [C, 9 * GRP], f32r,
                                    isOutput=False)
    outp = nc.declare_dram_parameter("out", [GRP, 1], f32, isOutput=True)

    seg4 = seg[:].rearrange("(h a) (w b) -> h a w b", a=4, b=4)  # [320,4,320,4]

    with ExitStack() as ctx:
        tc = ctx.enter_context(tile.TileContext(nc))

        consts = ctx.enter_context(tc.tile_pool(name="consts", bufs=1))
        xpool = ctx.enter_context(tc.tile_pool(name="xcrop", bufs=12))
        hpool = ctx.enter_context(tc.tile_pool(name="hbuf", bufs=8))
        msegp = ctx.enter_context(tc.tile_pool(name="mseg", bufs=3))
        bcep = ctx.enter_context(tc.tile_pool(name="bce", bufs=3))
        accp = ctx.enter_context(tc.tile_pool(name="acc", bufs=12))
        rp = ctx.enter_context(tc.tile_pool(name="rsum", bufs=1))
        cgp = ctx.enter_context(tc.tile_pool(name="coordg", bufs=4))

        c1p = ctx.enter_context(tc.tile_pool(name="c1psum", bufs=3, space="PSUM"))
        zpp = ctx.enter_context(tc.tile_pool(name="zpsum", bufs=3, space="PSUM"))
        lpp = ctx.enter_context(tc.tile_pool(name="lpsum", bufs=2, space="PSUM"))

        # ---- constants / weights into SBUF ----
        w1_sb = consts.tile([C, 9 * 256], f32r)
        # load the center-tap half-0 block first so the first conv1 matmul
        # is not gated by the full 1.2 MB weight transfer
        w1v = w1t[:].transpose([1, 0, 2])  # [ci, tap, co]
        nc.sync.dma_start(out=w1_sb[:, 4 * 256:4 * 256 + 128],
                          in_=w1v[:, 4:5, 0:128])
        nc.sync.dma_start(out=w1_sb[:, 0:4 * 256], in_=w1v[:, 0:4, :])
        nc.sync.dma_start(out=w1_sb[:, 4 * 256 + 128:5 * 256],
                          in_=w1v[:, 4:5, 128:256])
        nc.sync.dma_start(out=w1_sb[:, 5 * 256:], in_=w1v[:, 5:9, :])
        w2_sb = consts.tile([C, 18], f32r)
        nc.sync.dma_start(out=w2_sb[:], in_=w2t[:])
        b1_sb = consts.tile([C, 2], f32)
        nc.sync.dma_start(out=b1_sb[:], in_=b1t[:])
        b2_sb = consts.tile([C, 1], f32)
        nc.sync.dma_start(out=b2_sb[:], in_=b2t[:])
        e36_sb = consts.tile([C, 9 * GRP], f32r)
        nc.sync.dma_start(out=e36_sb[:], in_=e36[:])

        R4 = rp.tile([GRP, 1], f32)
        nc.any.memset(R4[:], 0.0)

        # f32 zeros used to zero-fill f32r tiles via DVE copy (walrus requires
        # fp32r matmul inputs to come from rounding producers; memset is not)
        zf_sb = consts.tile([C, NPAD], f32)
        nc.any.memset(zf_sb[:], 0.0)

        # persistent group Z tile: anchor j's 9 tap rows live at partition
        # 32j (DVE partition access must be 32-aligned); the other 23 rows
        # of each block stay zero forever so the stage-B unit columns that
        # multiply them contribute exact zeros (never NaN garbage)
        z_sbs = []
        for s in range(NSUB):
            z = consts.tile([C, NXP], f32r, name=f"z_sb{s}")
            nc.vector.tensor_copy(out=z[:], in_=zf_sb[:, 0:NXP])
            z_sbs.append(z)

        SP_ONLY = (mybir.EngineType.SP,)
        POOL_ONLY = (mybir.EngineType.Pool,)
        zchunks = [(0, 512), (512, 512)]
        TAP_ORDER = [4, 0, 1, 2, 3, 5, 6, 7, 8]

        with tc.For_i(0, NBODY, 1, staggered_reset=True,
                      hint_engines=(mybir.EngineType.PE,)) as g:
            NA = GRP * NSUB  # 8 anchors per body
            coords_g = cgp.tile([1, 2 * NA], i32, tag="cg")
            nc.sync.dma_start(out=coords_g[0:1, 0:NA],
                              in_=coords[0:1, ds(NA * g, NA)])
            nc.sync.dma_start(out=coords_g[0:1, NA:2 * NA],
                              in_=coords[0:1, ds(APC + NA * g, NA)])
            cls_s = []
            mseg_s = []
            for s in range(NSUB):
                cg = cgp.tile([GRP, 1], f32, tag=f"clsg{s}", name=f"cls_{s}")
                nc.sync.dma_start(out=cg[0:GRP, 0:1],
                                  in_=clsv[0:1, ds(NA * g + GRP * s, GRP)])
                cls_s.append(cg)
                mseg_s.append(msegp.tile([GRP, 1024], i32, tag=f"mseg{s}",
                                         name=f"mseg_{s}"))

            # issue all dynamic DMAs up front: feature crops from SP (HWDGE),
            # seg crops from Pool (SWDGE) — split across engines both for
            # queue parallelism and per-engine register-file headroom
            xts_ = []
            for a in range(NA):
                s, j = a // GRP, a % GRP
                yv = nc.values_load(
                    coords_g[0:1, a:a + 1], engines=SP_ONLY,
                    min_val=0, max_val=HF - CROP,
                    skip_runtime_bounds_check=True,
                )
                xv = nc.values_load(
                    coords_g[0:1, NA + a:NA + a + 1], engines=SP_ONLY,
                    min_val=0, max_val=WF - CROP,
                    skip_runtime_bounds_check=True,
                )
                yvp = nc.values_load(
                    coords_g[0:1, a:a + 1], engines=POOL_ONLY,
                    min_val=0, max_val=HF - CROP,
                    skip_runtime_bounds_check=True,
                )
                xvp = nc.values_load(
                    coords_g[0:1, NA + a:NA + a + 1], engines=POOL_ONLY,
                    min_val=0, max_val=WF - CROP,
                    skip_runtime_bounds_check=True,
                )

                # mask crop: seg[4*(y0+y), 4*(x0+x)] -> [1, 1024] int32
                nc.gpsimd.dma_start(
                    out=mseg_s[s][j:j + 1, 0:1024],
                    in_=seg4[ds(yvp, CROP), 0, ds(xvp, CROP), 0],
                )

                # feature crop into x-only padded rows (34-wide, cols 0 and
                # 33 zeroed; row edges handled by clipping the tap regions)
                xt = xpool.tile([C, NXP], f32r, tag="xc", name=f"xc_{a}")
                xts_.append(xt)
                xtv = xt[:].rearrange("p (h w) -> p h w", h=CROP)
                nc.vector.tensor_copy(
                    out=xtv[:, :, 0:WPAD:WPAD - 1],
                    in_=zf_sb[:, 0:2 * CROP].rearrange("p (a b) -> p a b", b=2),
                )
                nc.sync.dma_start(
                    out=xtv[:, :, 1:1 + CROP],
                    in_=feat[:, ds(yv, CROP), ds(xv, CROP)],
                )
                for v in (yv, xv, yvp, xvp):
                    for reg in v.val.handles:
                        nc.free_register(reg)

            for s in range(NSUB):
                z_sb = z_sbs[s]
                for j in range(GRP):
                    xv3 = xts_[s * GRP + j][:].rearrange("p (h w) -> p h w",
                                                         h=CROP)

                    # conv1 (3x3 SAME): x pad columns absorb dx shifts; dy row
                    # edges are clipped (center tap first so its start=True
                    # write covers every output element) + bias/relu
                    h_sb = []
                    for half in range(2):
                        h = hpool.tile([C, NPX], f32r, tag="hb",
                                       name=f"hb_{s}_{j}_{half}")
                        h_sb.append(h)
                        hv3 = h[:].rearrange("p (h w) -> p h w", h=CROP)
                        ps = [c1p.tile([C, 512], f32, tag="c1",
                                       name=f"c1_{s}_{j}_{half}_{nt}")
                              for nt in range(2)]
                        psv = [p[:].rearrange("p (h w) -> p h w", h=16)
                               for p in ps]
                        for t in TAP_ORDER:
                            dy, dx = t // 3, t % 3
                            lhsT = w1_sb[:, t * 256 + half * 128:
                                         t * 256 + half * 128 + 128]
                            for nt in range(2):
                                y0_, y1_ = 16 * nt, 16 * nt + 16
                                r0 = max(y0_, 1 - dy)
                                r1 = min(y1_, CROP + 1 - dy)
                                nc.tensor.matmul(
                                    psv[nt][:, r0 - y0_:r1 - y0_, :],
                                    lhsT,
                                    xv3[:, r0 + dy - 1:r1 + dy - 1,
                                        dx:dx + CROP],
                                    start=(t == 4),
                                    stop=(t == TAP_ORDER[-1]),
                                )
                        for nt in range(2):
                            nc.scalar.activation(
                                hv3[:, 16 * nt:16 * nt + 16, :],
                                ps[nt][:], AF.Relu,
                                bias=b1_sb[:, half:half + 1], scale=1.0,
                            )

                    # conv2 stage A: Z[m, q] = sum_ci h[ci, q] * W2[ci, m],
                    # stacked at partition 32j of this sub-group's Z tile
                    for qi, (q0, qn) in enumerate(zchunks):
                        zps = zpp.tile([16, 512], f32, tag="zp",
                                       name=f"zp_{s}_{j}_{qi}")
                        for half in range(2):
                            nc.tensor.matmul(
                                zps[0:9, 0:qn],
                                w2_sb[:, 9 * half:9 * half + 9],
                                h_sb[half][:, q0:q0 + qn],
                                start=(half == 0), stop=(half == 1),
                            )
                        zw = z_sb[:].rearrange("p (h w) -> p h w", h=CROP)
                        nc.vector.tensor_copy(
                            out=zw[32 * j:32 * j + 9,
                                   (q0 // 512) * 16:(q0 // 512) * 16 + 16,
                                   1:33],
                            in_=zps[0:9, 0:qn])

                zv3 = z_sb[:].rearrange("p (h w) -> p h w", h=CROP)

                # conv2 stage B for the sub-group's 4 anchors (K=105, M=4)
                KZ = 32 * (GRP - 1) + 9
                for nt in range(2):
                    lt = lpp.tile([GRP, 512], f32, tag="lp",
                                  name=f"lp_{s}_{nt}")
                    ltv = lt[:].rearrange("p (h w) -> p h w", h=16)
                    for t in TAP_ORDER:
                        dy, dx = t // 3, t % 3
                        y0_, y1_ = 16 * nt, 16 * nt + 16
                        r0 = max(y0_, 1 - dy)
                        r1 = min(y1_, CROP + 1 - dy)
                        nc.tensor.matmul(
                            ltv[0:GRP, r0 - y0_:r1 - y0_, :],
                            e36_sb[0:KZ, GRP * t:GRP * t + GRP],
                            zv3[0:KZ, r0 + dy - 1:r1 + dy - 1, dx:dx + CROP],
                            start=(t == 4), stop=(t == TAP_ORDER[-1]),
                        )

                    # tgt = (mask == cls) in f32 (small ints, exact)
                    mf = bcep.tile([GRP, 512], f32, tag="mf")
                    nc.vector.tensor_copy(
                        out=mf[:],
                        in_=mseg_s[s][0:GRP, 512 * nt:512 * nt + 512])
                    tgt = bcep.tile([GRP, 512], f32, tag="tgt")
                    nc.vector.tensor_scalar(
                        out=tgt[:], in0=mf[:],
                        scalar1=cls_s[s][0:GRP, 0:1], scalar2=None,
                        op0=OP.is_equal,
                    )
                    # stable softplus: relu(x) + ln(1 + exp(-|x|)), x = L+b2
                    ab = bcep.tile([GRP, 512], f32, tag="ab")
                    nc.scalar.activation(ab[:], lt[:], AF.Abs,
                                         bias=b2_sb[0:GRP, 0:1], scale=1.0)
                    ex = bcep.tile([GRP, 512], f32, tag="ex")
                    nc.scalar.activation(ex[:], ab[:], AF.Exp,
                                         bias=0.0, scale=-1.0)
                    sp = bcep.tile([GRP, 512], f32, tag="sp")
                    acc_ln = accp.tile([GRP, 1], f32, tag="acc")
                    nc.scalar.activation(sp[:], ex[:], AF.Ln,
                                         bias=1.0, scale=1.0,
                                         accum_out=acc_ln[:])
                    rl = bcep.tile([GRP, 512], f32, tag="rl")
                    acc_rl = accp.tile([GRP, 1], f32, tag="acc")
                    nc.scalar.activation(rl[:], lt[:], AF.Relu,
                                         bias=b2_sb[0:GRP, 0:1], scale=1.0,
                                         accum_out=acc_rl[:])
                    # (L + b2) * tgt with row-sum
                    lb = bcep.tile([GRP, 512], f32, tag="lb")
                    nc.vector.tensor_scalar(
                        out=lb[:], in0=lt[:], scalar1=b2_sb[0:GRP, 0:1],
                        scalar2=None, op0=OP.add,
                    )
                    xts = bcep.tile([GRP, 512], f32, tag="xts")
                    nc.vector.tensor_tensor(out=xts[:], in0=lb[:],
                                            in1=tgt[:], op=OP.mult)
                    acc_xt = accp.tile([GRP, 1], f32, tag="acc")
                    nc.vector.reduce_sum(acc_xt[:], xts[:],
                                         axis=mybir.AxisListType.X)
                    # R4 += acc_rl + acc_ln - acc_xt
                    dsum = accp.tile([GRP, 1], f32, tag="acc")
                    nc.vector.tensor_tensor(out=dsum[:], in0=acc_rl[:],
                                            in1=acc_ln[:], op=OP.add)
                    nc.vector.tensor_tensor(out=dsum[:], in0=dsum[:],
                                            in1=acc_xt[:], op=OP.subtract)
                    nc.vector.tensor_tensor(out=R4[:], in0=R4[:],
                                            in1=dsum[:], op=OP.add)

        out_sb = consts.tile([GRP, 1], f32)
        nc.vector.tensor_copy(out=out_sb[:], in_=R4[:])
        nc.sync.dma_start(out=outp[0:GRP, 0:1], in_=out_sb[:])

    nc.compile()
    return nc


def _get_program():
    if "nc" not in _cache:
        _cache["nc"] = _build_program()
    return _cache["nc"]


def kernel(feature_map, seg, anchors, labels, base_classes, W1, b1, W2, b2):
    global last_exec_time_ns, last_results
    import os
    from concourse.bass_utils import run_bass_kernel_spmd

    feature_map = np.ascontiguousarray(feature_map, dtype=np.float32)
    seg = np.ascontiguousarray(seg, dtype=np.int32)
    anchors = np.asarray(anchors, dtype=np.int32)
    labels = np.asarray(labels, dtype=np.int32)
    base_classes = np.asarray(base_classes, dtype=np.int32)
    W1 = np.asarray(W1, dtype=np.float32)
    b1 = np.asarray(b1, dtype=np.float32)
    W2 = np.asarray(W2, dtype=np.float32)
    b2 = np.asarray(b2, dtype=np.float32)

    # weight layouts for the device
    w1tr = np.ascontiguousarray(W1.transpose(2, 3, 1, 0).reshape(9, C, 256))
    w2tr = np.ascontiguousarray(
        W2[0].reshape(2, C, 9).transpose(1, 0, 2).reshape(C, 18))
    b1tr = np.ascontiguousarray(b1.reshape(2, C).T)
    b2tr = np.full((C, 1), b2[0], dtype=np.float32)
    e36v = np.zeros((C, 9 * GRP), dtype=np.float32)
    for t in range(9):
        for j in range(GRP):
            e36v[32 * j + t, GRP * t + j] = 1.0
    tgt_cls = base_classes[labels].astype(np.float32)  # [256]

    y0 = anchors[:, 2].astype(np.int32)
    x0 = anchors[:, 0].astype(np.int32)

    nc = _get_program()
    in_maps = []
    for c in range(NCORES):
        sl = slice(c * APC, (c + 1) * APC)
        coords = np.concatenate([y0[sl], x0[sl]]).reshape(1, 2 * APC)
        in_maps.append({
            "feat": feature_map,
            "seg": seg,
            "coords": np.ascontiguousarray(coords, dtype=np.int32),
            "clsv": np.ascontiguousarray(tgt_cls[sl].reshape(1, APC)),
            "w1t": w1tr,
            "w2t": w2tr,
            "b1t": b1tr,
            "b2t": b2tr,
            "e36": e36v,
        })

    trace = os.environ.get("BASS_KERNEL_TRACE", "0") == "1"
    try:
        rb = run_bass_kernel_spmd(nc, in_maps, list(range(NCORES)), trace=trace)
    except ModuleNotFoundError:
        rb = run_bass_kernel_spmd(nc, in_maps, list(range(NCORES)), trace=False)
    last_results = rb
    last_exec_time_ns = rb.exec_time_ns

    partials = [float(rb.results[c]["out"].sum(dtype=np.float64))
                for c in range(NCORES)]
    total = sum(partials) / CROP / CROP / (NANCH + 1e-10)
    return np.float32(total)



# revision 8
# speedup vs baseline: 1.7156x; 1.7156x over previous
"""Trainium2 Bass kernel for BinaryMaskPredictor (ragged anchors), fp8 rev.

Data-parallel over the 256 anchors: 32 anchors per NeuronCore on 8 cores.
feature_map / seg / conv weights are replicated; per-core anchor coords and
target classes are sharded.  Each core computes sum over its anchors of
sum_px BCE(logits, tgt); the host sums the 8x4 partials and normalizes.

Speed levers vs the fp32r baseline (425 us):
  * conv1 in fp8e4 DoubleRow (0.5 cyc/row).  Walrus requires the two
    moving K-tiles of a DoubleRow matmul to sit a multiple of 64 bytes
    apart, so the crop lives in a pitch-64 padded flat domain: taps
    (dy,dx) and (dy+1,dx) pair at stride exactly 64.  Six pair-matmuls
    (three real pairs + three zero-padded singles) replace 18 fp32r taps.
    Outputs are restricted to the 32 data rows (2048 f32 = two 2-bank
    PSUM tiles per half); pad columns inside the rows produce garbage
    that is never read.
  * stage A (Z = W2^T h) is one DoubleRow matmul per chunk contracting
    both ci halves at once (h half stride padded to 1216 = 64*19).
  * stage B stays plain fp8 (tap pairs on the pitch-34 z tile cannot be
    64-aligned) with interior-only [4,16,32] outputs.
  * BCE: sum(bce) = sum(ln(1+exp(L))) - sum(L*tgt); exp/ln on ACT with
    accumulate, tgt compare on GPSIMD (int32 seg vs f32 class id),
    mult + row-reduce on DVE.  b2 rides the center-tap row of the Z
    copy bias; b1 rides the conv1 relu bias; the fp8 weight scaling
    (W1*32, W2*64) is undone by the Z-copy scale 1/2048.
  * stage B / BCE of subgroup s are emitted interleaved into subgroup
    s+1's conv1 stream so the single-bank lt tile never stalls PE.
PSUM: c1 2x[128,1024] (4 banks) + zps [16,1216] (3) + lt [4,512] (1) = 8.
"""

import numpy as np
from contextlib import ExitStack

C = 128
HF = WF = 320
IMG = 1280
NANCH = 256
CROP = 32
P64 = 64                  # conv1 padded row pitch
P34 = 34                  # h / z padded row pitch
HLEN = 1216               # per-half h region (34*34=1156 padded to 64*19)
XD = 130                  # x-tile offset of crop pixel (0,0)
XLEN = 2368               # x tile length (reads reach 2241)
ZD = 70                   # z-tile offset of crop pixel (0,0)
ZLEN = 1232               # z tile length (35 guard + 34*34 + 35, padded)
NCORES = 8
APC = NANCH // NCORES     # 32 anchors per core
GRP = 4                   # anchors per stage-B subgroup
NSG = APC // GRP          # 8 subgroups per core
W1S = 32.0                # fp8 scale for W1 (+b1)
W2S = 64.0                # fp8 scale for W2
ZSC = 1.0 / (W1S * W2S)   # undo scaling at the Z copy
ACHUNKS = ((0, 512), (512, 512), (1024, HLEN - 1024))
# conv1 DoubleRow pairs (tap t=(dy,dx)=(t//3,t%3)): (t,t+3) at stride 64
C1PAIRS = [(0, 3), (1, 4), (2, 5), (6, None), (7, None), (8, None)]

_cache = {}
last_exec_time_ns = None
last_results = None


def _build_program():
    import concourse.bass as bass
    import concourse.tile as tile
    import concourse.mybir as mybir
    from concourse import bacc
    from concourse.bass import ds

    f32 = mybir.dt.float32
    f8 = mybir.dt.float8e4
    i32 = mybir.dt.int32
    AF = mybir.ActivationFunctionType
    OP = mybir.AluOpType
    PM = mybir.MatmulPerfMode

    nc = bacc.Bacc("TRN2", target_bir_lowering=False, debug=False,
                   num_devices=NCORES)

    feat = nc.declare_dram_parameter("feat", [C, HF, WF], f8, isOutput=False)
    seg = nc.declare_dram_parameter("seg", [IMG, IMG], i32, isOutput=False)
    coords = nc.declare_dram_parameter("coords", [1, 2 * APC], i32,
                                       isOutput=False)
    clsv = nc.declare_dram_parameter("clsv", [GRP, NSG], f32, isOutput=False)
    w1t = nc.declare_dram_parameter("w1t", [C, 3072], f8, isOutput=False)
    w2t = nc.declare_dram_parameter("w2t", [C, 32], f8, isOutput=False)
    e9t = nc.declare_dram_parameter("e9t", [C, 36], f8, isOutput=False)
    b1t = nc.declare_dram_parameter("b1t", [C, 2], f32, isOutput=False)
    b2t = nc.declare_dram_parameter("b2t", [C, 1], f32, isOutput=False)
    outp = nc.declare_dram_parameter("out", [GRP, 1], f32, isOutput=True)

    seg4 = seg[:].rearrange("(h a) (w b) -> h a w b", a=4, b=4)

    SP_ONLY = (mybir.EngineType.SP,)
    ACT_ONLY = (mybir.EngineType.Activation,)

    XR, HR, ZR, MR = 4, 3, 2, 2

    def view(t, off, dims):
        v = t[:]
        return bass.AP(v.tensor, v.offset + off, [list(v.ap[0])] + dims)

    def pview(t, off, dims, nparts):
        v = t[:]
        return bass.AP(v.tensor, v.offset + off,
                       [[v.ap[0][0], nparts]] + dims)

    with ExitStack() as ctx:
        tc = ctx.enter_context(tile.TileContext(nc))

        consts = ctx.enter_context(tc.tile_pool(name="consts", bufs=1))

        c1p = ctx.enter_context(tc.tile_pool(name="c1p", bufs=2, space="PSUM"))
        zpp = ctx.enter_context(tc.tile_pool(name="zpp", bufs=1, space="PSUM"))
        ltp = ctx.enter_context(tc.tile_pool(name="ltp", bufs=1, space="PSUM"))

        # ---- weights / constants ----
        w1_sb = consts.tile([C, 3072], f8)
        nc.sync.dma_start(out=w1_sb[:, 0:512], in_=w1t[:, 0:512])
        nc.sync.dma_start(out=w1_sb[:, 512:3072], in_=w1t[:, 512:3072])
        w2_sb = consts.tile([C, 32], f8)
        nc.sync.dma_start(out=w2_sb[:], in_=w2t[:])
        e9_sb = consts.tile([C, 36], f8)
        nc.sync.dma_start(out=e9_sb[:], in_=e9t[:])
        b1_sb = consts.tile([C, 2], f32)
        nc.sync.dma_start(out=b1_sb[:], in_=b1t[:])
        b2_sb = consts.tile([C, 1], f32)
        nc.sync.dma_start(out=b2_sb[:], in_=b2t[:])
        cls_sb = consts.tile([GRP, NSG], f32)
        nc.sync.dma_start(out=cls_sb[:], in_=clsv[:])
        coords_sb = consts.tile([1, 2 * APC], i32)
        nc.sync.dma_start(out=coords_sb[:], in_=coords[:])

        # persistent tiles; pads/guards zeroed once (DMA / copies only ever
        # write interior positions, so the zero borders survive reuse)
        x_tiles = [consts.tile([C, XLEN], f8, name=f"xt{i}")
                   for i in range(XR)]
        z_tiles = [consts.tile([C, ZLEN], f8, name=f"zt{i}")
                   for i in range(ZR)]
        h_tiles = [consts.tile([C, 2 * HLEN], f8, name=f"ht{i}")
                   for i in range(HR)]
        for t in x_tiles + z_tiles + h_tiles:
            nc.any.memset(t[:], 0.0)
        mseg_tiles = [consts.tile([GRP, 1024], i32, name=f"mt{i}")
                      for i in range(MR)]
        tgt_tiles = [consts.tile([GRP, 1024], f32, name=f"tt{i}")
                     for i in range(2)]
        e_tiles = [consts.tile([GRP, 1024], f32, name=f"et{i}")
                   for i in range(2)]
        xts_tiles = [consts.tile([GRP, 1024], f32, name=f"xx{i}")
                     for i in range(2)]
        lnout_tiles = [consts.tile([GRP, 1024], f32, name=f"lo{i}")
                       for i in range(2)]
        acc_sp = consts.tile([GRP, NSG], f32)
        acc_xt = consts.tile([GRP, NSG], f32)

        zps = zpp.tile([16, HLEN], f32)    # stage-A out, ring 1 (3 banks)
        ltt = ltp.tile([GRP, 512], f32)    # stage-B out, ring 1 (1 bank)

        def emit_front(a):
            """conv1 + relu + stage A + z copy for anchor a."""
            sg, j = divmod(a, GRP)
            xt = x_tiles[a % XR]
            ht = h_tiles[a % HR]
            z_sb = z_tiles[sg % ZR]
            mseg_t = mseg_tiles[sg % MR]

            # dynamic crop DMAs
            yv = nc.values_load(coords_sb[0:1, a:a + 1], engines=SP_ONLY,
                                min_val=0, max_val=HF - CROP,
                                skip_runtime_bounds_check=True)
            xv = nc.values_load(coords_sb[0:1, APC + a:APC + a + 1],
                                engines=SP_ONLY, min_val=0,
                                max_val=WF - CROP,
                                skip_runtime_bounds_check=True)
            nc.sync.dma_start(
                out=view(xt, XD, [[P64, CROP], [1, CROP]]),
                in_=feat[:, ds(yv, CROP), ds(xv, CROP)],
            )
            for v in (yv, xv):
                for reg in v.val.handles:
                    nc.free_register(reg)

            ya = nc.values_load(coords_sb[0:1, a:a + 1], engines=ACT_ONLY,
                                min_val=0, max_val=HF - CROP,
                                skip_runtime_bounds_check=True)
            xa = nc.values_load(coords_sb[0:1, APC + a:APC + a + 1],
                                engines=ACT_ONLY, min_val=0,
                                max_val=WF - CROP,
                                skip_runtime_bounds_check=True)
            nc.scalar.dma_start(
                out=mseg_t[j:j + 1, 0:1024],
                in_=seg4[ds(ya, CROP), 0, ds(xa, CROP), 0],
            )
            for v in (ya, xa):
                for reg in v.val.handles:
                    nc.free_register(reg)

            # conv1: out rows R34 in 1..32 as two 2-bank tiles per half;
            # flat out index f' = 64*(R34-1) + C64; moving operand reads
            # x at f' + 64 + 64*dy + dx (pairs at stride 64)
            for hf in range(2):
                for hr in range(2):   # rows 1..16 / 17..32
                    ct = c1p.tile([C, 1024], f32, tag="c1",
                                  name=f"c1_{a}_{hf}_{hr}")
                    base = 1024 * hr
                    for ci in range(2):      # two 512 bank chunks
                        coff = base + 512 * ci
                        for p, (tA, tB) in enumerate(C1PAIRS):
                            dyA, dxA = tA // 3, tA % 3
                            offA = XD - P64 - 2 + coff + P64 * dyA + dxA
                            lhs = view(w1_sb, p * 512 + hf * 256,
                                       [[128, 2], [1, 128]])
                            rhs = view(xt, offA, [[P64, 2], [1, 512]])
                            nc.tensor.matmul(ct[:, 512 * ci:512 * ci + 512],
                                             lhs, rhs,
                                             start=(p == 0), stop=(p == 5),
                                             perf_mode=PM.DoubleRow)
                    # relu: 16 rows x 34 cols -> h (pitch 34), fp8
                    hq = hf * HLEN + P34 * (1 + 16 * hr)
                    hview = view(ht, hq, [[P34, 16], [1, P34]])
                    cview = view(ct, 0, [[P64, 16], [1, P34]])
                    if hf == 0:
                        nc.scalar.activation(hview, cview, AF.Relu,
                                             bias=b1_sb[:, hf:hf + 1],
                                             scale=1.0)
                    else:
                        nc.vector.tensor_scalar(
                            out=hview, in0=cview,
                            scalar1=b1_sb[:, hf:hf + 1], scalar2=0.0,
                            op0=OP.add, op1=OP.max)

            # stage A: Z[9, q] over h flat (DR over ci halves)
            for coff, csz in ACHUNKS:
                rhs = view(ht, coff, [[HLEN, 2], [1, csz]])
                lhs = view(w2_sb, 0, [[16, 2], [1, 9]])
                nc.tensor.matmul(zps[0:9, coff:coff + csz], lhs, rhs,
                                 start=True, stop=True,
                                 perf_mode=PM.DoubleRow)

            # Z copy: interior rows/cols only, *1/2048, +b2 on center tap
            zin = pview(zps, P34 + 1, [[P34, CROP], [1, CROP]], 9)
            zout = view(z_sb, ZD, [[P34, CROP], [1, CROP]])
            zout = bass.AP(zout.tensor,
                           zout.offset + 32 * j * z_sb[:].ap[0][0],
                           [[zout.ap[0][0], 9]] + list(zout.ap)[1:])
            if a % 2 == 0:
                nc.scalar.activation(zout, zin, AF.Identity,
                                     bias=b2_sb[32 * j:32 * j + 9, 0:1],
                                     scale=ZSC)
            else:
                nc.vector.tensor_scalar(
                    out=zout, in0=zin, scalar1=ZSC,
                    scalar2=b2_sb[32 * j:32 * j + 9, 0:1],
                    op0=OP.mult, op1=OP.add)

        def emit_back(sg, nt):
            """stage B chunk nt + its BCE for subgroup sg."""
            z_sb = z_tiles[sg % ZR]
            mseg_t = mseg_tiles[sg % MR]
            tgt = tgt_tiles[sg % 2]
            et = e_tiles[sg % 2]
            xts = xts_tiles[sg % 2]

            if nt == 0:
                nc.gpsimd.tensor_scalar(
                    out=tgt[:], in0=mseg_t[:],
                    scalar1=cls_sb[0:GRP, sg:sg + 1], scalar2=None,
                    op0=OP.is_equal)

            y0 = 16 * nt
            for t in range(9):
                dy, dx = t // 3, t % 3
                lhs = pview(e9_sb, t * 4, [[1, 4]], 105)
                rhs = pview(z_sb, ZD + P34 * (y0 + dy - 1) + dx - 1,
                            [[P34, 16], [1, CROP]], 105)
                nc.tensor.matmul(ltt[:], lhs, rhs,
                                 start=(t == 0), stop=(t == 8))

            nc.scalar.activation(et[:, 512 * nt:512 * nt + 512], ltt[:],
                                 AF.Exp, bias=0.0, scale=1.0)
            nc.vector.tensor_tensor(
                out=xts[:, 512 * nt:512 * nt + 512], in0=ltt[:],
                in1=tgt[:, 512 * nt:512 * nt + 512], op=OP.mult)
            if nt == 1:
                nc.scalar.activation(lnout_tiles[sg % 2][:], et[:], AF.Ln,
                                     bias=1.0, scale=1.0,
                                     accum_out=acc_sp[:, sg:sg + 1])
                nc.vector.reduce_sum(acc_xt[:, sg:sg + 1], xts[:],
                                     axis=mybir.AxisListType.X)

        # interleaved emission: stage B / BCE of subgroup sg rides inside
        # subgroup sg+1's conv1 stream
        pending = []   # (sg, nt) stage-B chunks not yet emitted
        for a in range(APC):
            emit_front(a)
            if pending:
                emit_back(*pending.pop(0))
            if a % GRP == GRP - 1:
                pending += [(a // GRP, 0), (a // GRP, 1)]
        for args in pending:
            emit_back(*args)

        # ---- final: out[j] = sum_sg (sp - xt) ----
        diff = consts.tile([GRP, NSG], f32)
        nc.vector.tensor_tensor(out=diff[:], in0=acc_sp[:], in1=acc_xt[:],
                                op=OP.subtract)
        osb = consts.tile([GRP, 1], f32)
        nc.vector.reduce_sum(osb[:], diff[:], axis=mybir.AxisListType.X)
        nc.sync.dma_start(out=outp[0:GRP, 0:1], in_=osb[:])

    nc.compile()
    return nc


def _get_program():
    if "nc" not in _cache:
        _cache["nc"] = _build_program()
    return _cache["nc"]


def host_inputs(feature_map, seg, anchors, labels, base_classes, W1, b1, W2,
                b2):
    """Device-layout input maps for all cores (shared tensors prepared once)."""
    import ml_dtypes
    f8 = ml_dtypes.float8_e4m3

    feature_map = np.ascontiguousarray(feature_map, dtype=np.float32)
    seg = np.ascontiguousarray(seg, dtype=np.int32)
    anchors = np.asarray(anchors, dtype=np.int32)
    labels = np.asarray(labels, dtype=np.int32)
    base_classes = np.asarray(base_classes, dtype=np.int32)
    W1 = np.asarray(W1, dtype=np.float32)
    b1 = np.asarray(b1, dtype=np.float32)
    W2 = np.asarray(W2, dtype=np.float32)
    b2 = np.asarray(b2, dtype=np.float32)

    feat8 = np.ascontiguousarray(feature_map.astype(f8))

    # w1: [ci, pair(6) x half(2) x tile(2) x co(128)], pairs (t,t+3)/(t,0)
    w1h = np.zeros((C, 6, 2, 2, 128), dtype=np.float32)
    for p, (tA, tB) in enumerate(C1PAIRS):
        for hf in range(2):
            w1h[:, p, hf, 0, :] = (
                W1[hf * 128:(hf + 1) * 128, :, tA // 3, tA % 3].T * W1S)
            if tB is not None:
                w1h[:, p, hf, 1, :] = (
                    W1[hf * 128:(hf + 1) * 128, :, tB // 3, tB % 3].T * W1S)
    w1h = np.ascontiguousarray(w1h.reshape(C, 3072).astype(f8))

    # w2: [ci, tile(2 halves) x 16 (9 taps + pad)]
    w2h = np.zeros((C, 2, 16), dtype=np.float32)
    for hf in range(2):
        w2h[:, hf, 0:9] = W2[0, hf * C:(hf + 1) * C].reshape(C, 9) * W2S
    w2h = np.ascontiguousarray(w2h.reshape(C, 32).astype(f8))

    # e9: unit columns, [part, tap(9) x anchor(4)]
    e9 = np.zeros((C, 9, GRP), dtype=np.float32)
    for t in range(9):
        for j in range(GRP):
            e9[32 * j + t, t, j] = 1.0
    e9 = np.ascontiguousarray(e9.reshape(C, 36).astype(f8))

    b1h = np.ascontiguousarray(b1.reshape(2, C).T * W1S)
    b2h = np.zeros((C, 1), dtype=np.float32)
    for j in range(GRP):
        b2h[32 * j + 4, 0] = b2[0]

    tgt_cls = base_classes[labels].astype(np.float32)
    y0 = anchors[:, 2].astype(np.int32)
    x0 = anchors[:, 0].astype(np.int32)

    in_maps = []
    for c in range(NCORES):
        sl = slice(c * APC, (c + 1) * APC)
        coords = np.concatenate([y0[sl], x0[sl]]).reshape(1, 2 * APC)
        clsv = np.ascontiguousarray(tgt_cls[sl].reshape(NSG, GRP).T)
        in_maps.append({
            "feat": feat8,
            "seg": seg,
            "coords": np.ascontiguousarray(coords, dtype=np.int32),
            "clsv": clsv,
            "w1t": w1h,
            "w2t": w2h,
            "e9t": e9,
            "b1t": b1h,
            "b2t": b2h,
        })
    return in_maps


def kernel(feature_map, seg, anchors, labels, base_classes, W1, b1, W2, b2):
    global last_exec_time_ns, last_results
    import os
    from concourse.bass_utils import run_bass_kernel_spmd

    in_maps = host_inputs(feature_map, seg, anchors, labels, base_classes,
                          W1, b1, W2, b2)
    nc = _get_program()

    trace = os.environ.get("BASS_KERNEL_TRACE", "0") == "1"
    try:
        rb = run_bass_kernel_spmd(nc, in_maps, list(range(NCORES)),
                                  trace=trace)
    except ModuleNotFoundError:
        rb = run_bass_kernel_spmd(nc, in_maps, list(range(NCORES)),
                                  trace=False)
    last_results = rb
    last_exec_time_ns = rb.exec_time_ns

    partials = [float(rb.results[c]["out"].sum(dtype=np.float64))
                for c in range(NCORES)]
    total = sum(partials) / CROP / CROP / (NANCH + 1e-10)
    return np.float32(total)


# revision 12
# speedup vs baseline: 1.8452x; 1.0756x over previous
"""Trainium2 Bass kernel for BinaryMaskPredictor (ragged anchors), fp8 rev 2.

Data-parallel over the 256 anchors: 32 anchors per NeuronCore on 8 cores.
feature_map / seg / conv weights are replicated; per-core anchor coords and
target classes are sharded.  Each core computes sum over its anchors of
sum_px BCE(logits, tgt); the host sums the 8x4 partials and normalizes.

Structure (all matmuls fp8e4; DoubleRow = 0.5 cyc/row):
  * Crops live in a DENSE pitch-32 domain (no pad columns).  Walrus
    requires DoubleRow K-tiles 64 bytes apart, so taps pair as
    (dy, dx)+(dy+2, dx): three real pairs (t0,t6),(t1,t7),(t2,t8) plus
    three zero-padded singles (t3),(t4),(t5).  Vertical padding is two
    zero rows above/below the crop.  Horizontal SAME-padding is fixed up
    afterwards: a dx=0 tap wrongly reads the previous row's col 31 at
    out col 0 (and dx=2 the next row's col 0 at out col 31); per tap a
    tiny negated-weight matmul on the same addresses cancels the bogus
    contribution exactly.  1024 true outputs per half = 2 PSUM banks.
  * h is dense [128, 2*1024] fp8 -> stage A (Z = W2^T h) is one DoubleRow
    matmul per 512-chunk contracting both ci halves (half stride 1024).
  * Z copies flat [9,1024] into a dense pitch-32 z tile; stage B uses the
    same pair+fixup scheme with unit/negated-unit e-columns (K=105,
    anchors stacked at partition 32j).
  * BCE: sum(bce) = sum(ln(1+exp(L))) - sum(L*tgt); exp/ln on ACT with
    accumulate, tgt compare on GPSIMD (int32 seg vs f32 class id),
    mult + row-reduce on DVE.  b2 rides the center-tap row of the Z copy
    bias; b1 rides the conv1 relu bias; fp8 weight scaling (W1*32,
    W2*64) is undone by the Z-copy scale 1/2048.
  * stage B / BCE of subgroup s are emitted interleaved into subgroup
    s+1's conv1 stream so the 2-bank lt tile never stalls PE.
PSUM: c1 2x[128,1024] (4 banks) + zps [16,1024] (2) + lt [4,1024] (2) = 8.
"""

import numpy as np
from contextlib import ExitStack

C = 128
HF = WF = 320
IMG = 1280
NANCH = 256
CROP = 32
XD = 64                   # x/z tile offset of crop pixel (0,0) (2 pad rows)
XLEN = 1160               # 64 + 32*34 + tail slack
NCORES = 8
APC = NANCH // NCORES     # 32 anchors per core
GRP = 4                   # anchors per stage-B subgroup
NSG = APC // GRP          # 8 subgroups per core
W1S = 32.0                # fp8 scale for W1 (+b1)
W2S = 64.0                # fp8 scale for W2
ZSC = 1.0 / (W1S * W2S)   # undo scaling at the Z copy
# DoubleRow pairs (tap t=(dy,dx)=(t//3,t%3)): (t, t+6) at stride 64
C1PAIRS = [(0, 6), (1, 7), (2, 8), (3, None), (4, None), (5, None)]
FIXTAPS = [0, 3, 6, 2, 5, 8]   # dx=0 taps fix col 0; dx=2 taps fix col 31

_cache = {}
last_exec_time_ns = None
last_results = None


def _build_program():
    import concourse.bass as bass
    import concourse.tile as tile
    import concourse.mybir as mybir
    from concourse import bacc
    from concourse.bass import ds

    f32 = mybir.dt.float32
    f8 = mybir.dt.float8e4
    i32 = mybir.dt.int32
    AF = mybir.ActivationFunctionType
    OP = mybir.AluOpType
    PM = mybir.MatmulPerfMode

    nc = bacc.Bacc("TRN2", target_bir_lowering=False, debug=False,
                   num_devices=NCORES)

    feat = nc.declare_dram_parameter("feat", [C, HF, WF], f8, isOutput=False)
    seg = nc.declare_dram_parameter("seg", [IMG, IMG], i32, isOutput=False)
    coords = nc.declare_dram_parameter("coords", [1, 2 * APC], i32,
                                       isOutput=False)
    clsv = nc.declare_dram_parameter("clsv", [GRP, NSG], f32, isOutput=False)
    w1t = nc.declare_dram_parameter("w1t", [C, 3072], f8, isOutput=False)
    w2t = nc.declare_dram_parameter("w2t", [C, 32], f8, isOutput=False)
    e9t = nc.declare_dram_parameter("e9t", [C, 192], f8, isOutput=False)
    b1t = nc.declare_dram_parameter("b1t", [C, 2], f32, isOutput=False)
    b2t = nc.declare_dram_parameter("b2t", [C, 1], f32, isOutput=False)
    outp = nc.declare_dram_parameter("out", [GRP, 1], f32, isOutput=True)

    seg4 = seg[:].rearrange("(h a) (w b) -> h a w b", a=4, b=4)

    SP_ONLY = (mybir.EngineType.SP,)
    ACT_ONLY = (mybir.EngineType.Activation,)

    XR, HR, ZR, MR = 4, 3, 2, 2

    def view(t, off, dims, nparts=None):
        v = t[:]
        p0 = list(v.ap[0]) if nparts is None else [v.ap[0][0], nparts]
        return bass.AP(v.tensor, v.offset + off, [p0] + dims)

    with ExitStack() as ctx:
        tc = ctx.enter_context(tile.TileContext(nc))

        consts = ctx.enter_context(tc.tile_pool(name="consts", bufs=1))

        c1p = ctx.enter_context(tc.tile_pool(name="c1p", bufs=2, space="PSUM"))
        zpp = ctx.enter_context(tc.tile_pool(name="zpp", bufs=1, space="PSUM"))
        ltp = ctx.enter_context(tc.tile_pool(name="ltp", bufs=1, space="PSUM"))
        sdp = ctx.enter_context(tc.tile_pool(name="sdp", bufs=1, space="PSUM"))

        # ---- weights / constants ----
        w1_sb = consts.tile([C, 3072], f8)
        nc.sync.dma_start(out=w1_sb[:, 0:512], in_=w1t[:, 0:512])
        nc.sync.dma_start(out=w1_sb[:, 512:3072], in_=w1t[:, 512:3072])
        w2_sb = consts.tile([C, 32], f8)
        nc.sync.dma_start(out=w2_sb[:], in_=w2t[:])
        e9_sb = consts.tile([C, 192], f8)
        nc.sync.dma_start(out=e9_sb[:], in_=e9t[:])
        b1_sb = consts.tile([C, 2], f32)
        nc.sync.dma_start(out=b1_sb[:], in_=b1t[:])
        b2_sb = consts.tile([C, 1], f32)
        nc.sync.dma_start(out=b2_sb[:], in_=b2t[:])
        cls_sb = consts.tile([GRP, NSG], f32)
        nc.sync.dma_start(out=cls_sb[:], in_=clsv[:])
        coords_sb = consts.tile([1, 2 * APC], i32)
        nc.sync.dma_start(out=coords_sb[:], in_=coords[:])

        # persistent tiles; pad rows zeroed once (DMA / copies only ever
        # write the crop interior, so the zero pads survive reuse)
        x_tiles = [consts.tile([C, XLEN], f8, name=f"xt{i}")
                   for i in range(XR)]
        z_tiles = [consts.tile([C, XLEN], f8, name=f"zt{i}")
                   for i in range(ZR)]
        for t in x_tiles + z_tiles:
            nc.any.memset(t[:], 0.0)
        h_tiles = [consts.tile([C, 2048], f8, name=f"ht{i}")
                   for i in range(HR)]
        mseg_tiles = [consts.tile([GRP, 1024], i32, name=f"mt{i}")
                      for i in range(MR)]
        tgt_tiles = [consts.tile([GRP, 1024], f32, name=f"tt{i}")
                     for i in range(2)]
        e_tiles = [consts.tile([GRP, 1024], f32, name=f"et{i}")
                   for i in range(2)]
        xts_tiles = [consts.tile([GRP, 1024], f32, name=f"xx{i}")
                     for i in range(2)]
        lnout_tiles = [consts.tile([GRP, 1024], f32, name=f"lo{i}")
                       for i in range(2)]
        acc_sp = consts.tile([GRP, NSG], f32)
        acc_xt = consts.tile([GRP, NSG], f32)

        zps = zpp.tile([16, 1024], f32)    # stage-A out, ring 1 (2 banks)
        ltt = ltp.tile([GRP, 512], f32)    # stage-B out chunk, ring 1 (1 bank)
        side = sdp.tile([C, 192], f32)     # edge-column psum, ring 1 (1 bank)
        # single-tap lhsT offsets in the pair layouts: tap -> (pair, tile)
        def tap_pt(t):
            if t < 3:
                return t, 0
            if t >= 6:
                return t - 6, 1
            return t, 0

        def emit_front(a):
            """conv1 + relu + stage A + z copy for anchor a."""
            sg, j = divmod(a, GRP)
            xt = x_tiles[a % XR]
            ht = h_tiles[a % HR]
            z_sb = z_tiles[sg % ZR]
            mseg_t = mseg_tiles[sg % MR]

            # dynamic crop DMAs
            yv = nc.values_load(coords_sb[0:1, a:a + 1], engines=SP_ONLY,
                                min_val=0, max_val=HF - CROP,
                                skip_runtime_bounds_check=True)
            xv = nc.values_load(coords_sb[0:1, APC + a:APC + a + 1],
                                engines=SP_ONLY, min_val=0,
                                max_val=WF - CROP,
                                skip_runtime_bounds_check=True)
            nc.sync.dma_start(
                out=view(xt, XD, [[CROP, CROP], [1, CROP]]),
                in_=feat[:, ds(yv, CROP), ds(xv, CROP)],
            )
            for v in (yv, xv):
                for reg in v.val.handles:
                    nc.free_register(reg)

            ya = nc.values_load(coords_sb[0:1, a:a + 1], engines=ACT_ONLY,
                                min_val=0, max_val=HF - CROP,
                                skip_runtime_bounds_check=True)
            xa = nc.values_load(coords_sb[0:1, APC + a:APC + a + 1],
                                engines=ACT_ONLY, min_val=0,
                                max_val=WF - CROP,
                                skip_runtime_bounds_check=True)
            nc.scalar.dma_start(
                out=mseg_t[j:j + 1, 0:1024],
                in_=seg4[ds(ya, CROP), 0, ds(xa, CROP), 0],
            )
            for v in (ya, xa):
                for reg in v.val.handles:
                    nc.free_register(reg)

            # conv1: dense out f = 32*y + x; tap (dy,dx) reads the x tile
            # at f + 32*dy + dx + XD - 33; pairs (t, t+6) sit 64 apart
            for hf in range(2):
                ct = c1p.tile([C, 1024], f32, tag="c1", name=f"c1_{a}_{hf}")
                for ci in range(2):
                    coff = 512 * ci
                    for p, (tA, tB) in enumerate(C1PAIRS):
                        dyA, dxA = tA // 3, tA % 3
                        offA = XD - 33 + coff + 32 * dyA + dxA
                        lhs = view(w1_sb, p * 512 + hf * 256,
                                   [[128, 2], [1, 128]])
                        rhs = view(xt, offA, [[64, 2], [1, 512]])
                        nc.tensor.matmul(ct[:, coff:coff + 512], lhs, rhs,
                                         start=(p == 0), stop=(p == 5),
                                         perf_mode=PM.DoubleRow)
                # edge columns 0/31 computed clean in the side tile
                # (main-pass values there read across rows; overwritten in h)
                for ei, ocol in enumerate((0, 31)):
                    sbase = hf * 64 + ei * 32
                    taps = [t for t in range(9)
                            if (t % 3 >= 1 if ocol == 0 else t % 3 <= 1)]
                    for ti, t in enumerate(taps):
                        dy, dx = t // 3, t % 3
                        rhso = XD + 32 * (dy - 1) + (dx - 1 + ocol)
                        p, tl = tap_pt(t)
                        lhs = view(w1_sb, p * 512 + hf * 256 + tl * 128,
                                   [[1, 128]])
                        rhs = view(xt, rhso, [[32, CROP]])
                        nc.tensor.matmul(side[:, sbase:sbase + 32], lhs, rhs,
                                         start=(ti == 0),
                                         stop=(ti == len(taps) - 1),
                                         skip_group_check=True)
                # bias + relu -> dense fp8 h (edge cols overwritten after)
                hview = ht[:, hf * 1024:hf * 1024 + 1024]
                if hf == 0:
                    nc.scalar.activation(hview, ct[:], AF.Relu,
                                         bias=b1_sb[:, hf:hf + 1], scale=1.0)
                else:
                    nc.vector.tensor_scalar(
                        out=hview, in0=ct[:],
                        scalar1=b1_sb[:, hf:hf + 1], scalar2=0.0,
                        op0=OP.add, op1=OP.max)
                for ei, ocol in enumerate((0, 31)):
                    sbase = hf * 64 + ei * 32
                    hcol = view(ht, hf * 1024 + ocol, [[CROP, CROP]])
                    if hf == 0:
                        nc.scalar.activation(hcol, side[:, sbase:sbase + 32],
                                             AF.Relu,
                                             bias=b1_sb[:, hf:hf + 1],
                                             scale=1.0)
                    else:
                        nc.vector.tensor_scalar(
                            out=hcol, in0=side[:, sbase:sbase + 32],
                            scalar1=b1_sb[:, hf:hf + 1], scalar2=0.0,
                            op0=OP.add, op1=OP.max)

            # stage A: Z[9, f] over dense h (DR over ci halves, stride 1024)
            for ci in range(2):
                coff = 512 * ci
                rhs = view(ht, coff, [[1024, 2], [1, 512]])
                lhs = view(w2_sb, 0, [[16, 2], [1, 9]])
                nc.tensor.matmul(zps[0:9, coff:coff + 512], lhs, rhs,
                                 start=True, stop=True,
                                 perf_mode=PM.DoubleRow)

            # Z copy: flat [9,1024], *1/2048, +b2 on center-tap row
            zout = view(z_sb, XD + 32 * j * XLEN, [[1, 1024]], 9)
            if a % 2 == 0:
                nc.scalar.activation(zout, zps[0:9, 0:1024], AF.Identity,
                                     bias=b2_sb[32 * j:32 * j + 9, 0:1],
                                     scale=ZSC)
            else:
                nc.vector.tensor_scalar(
                    out=zout, in0=zps[0:9, 0:1024], scalar1=ZSC,
                    scalar2=b2_sb[32 * j:32 * j + 9, 0:1],
                    op0=OP.mult, op1=OP.add)

        def emit_back(sg, nt):
            """stage B chunk nt (512 outs) + its BCE for subgroup sg."""
            z_sb = z_tiles[sg % ZR]
            mseg_t = mseg_tiles[sg % MR]
            tgt = tgt_tiles[sg % 2]
            et = e_tiles[sg % 2]
            xts = xts_tiles[sg % 2]

            if nt == 0:
                nc.gpsimd.tensor_scalar(
                    out=tgt[:], in0=mseg_t[:],
                    scalar1=cls_sb[0:GRP, sg:sg + 1], scalar2=None,
                    op0=OP.is_equal)

            coff = 512 * nt
            for p, (tA, tB) in enumerate(C1PAIRS):
                dyA, dxA = tA // 3, tA % 3
                offA = XD - 33 + coff + 32 * dyA + dxA
                lhs = view(e9_sb, p * 32, [[16, 2], [1, 4]], 105)
                rhs = view(z_sb, offA, [[64, 2], [1, 512]], 105)
                nc.tensor.matmul(ltt[:], lhs, rhs,
                                 start=(p == 0), stop=(p == 5),
                                 perf_mode=PM.DoubleRow)

            if nt == 0:
                # clean edge columns of the logits into the side tile
                for ei, ocol in enumerate((0, 31)):
                    sbase = 128 + ei * 32
                    taps = [t for t in range(9)
                            if (t % 3 >= 1 if ocol == 0 else t % 3 <= 1)]
                    for ti, t in enumerate(taps):
                        dy, dx = t // 3, t % 3
                        rhso = XD + 32 * (dy - 1) + (dx - 1 + ocol)
                        p, tl = tap_pt(t)
                        lhs = view(e9_sb, p * 32 + tl * 16, [[1, 4]], 105)
                        rhs = view(z_sb, rhso, [[32, CROP]], 105)
                        nc.tensor.matmul(side[0:GRP, sbase:sbase + 32],
                                         lhs, rhs,
                                         start=(ti == 0),
                                         stop=(ti == len(taps) - 1),
                                         skip_group_check=True)
                for ei, ocol in enumerate((0, 31)):
                    sbase = 128 + ei * 32
                    ecol = view(et, ocol, [[CROP, CROP]])
                    nc.scalar.activation(ecol, side[0:GRP, sbase:sbase + 32],
                                         AF.Exp, bias=0.0, scale=1.0)
                    xcol = view(xts, ocol, [[CROP, CROP]])
                    tcol = view(tgt, ocol, [[CROP, CROP]])
                    nc.vector.tensor_tensor(
                        out=xcol, in0=side[0:GRP, sbase:sbase + 32],
                        in1=tcol, op=OP.mult)

            # interior columns 1..30 from the chunk psum
            lint = view(ltt, 1, [[CROP, 16], [1, 30]])
            eint = view(et, coff + 1, [[CROP, 16], [1, 30]])
            nc.scalar.activation(eint, lint, AF.Exp, bias=0.0, scale=1.0)
            xint = view(xts, coff + 1, [[CROP, 16], [1, 30]])
            tint = view(tgt, coff + 1, [[CROP, 16], [1, 30]])
            nc.vector.tensor_tensor(out=xint, in0=lint, in1=tint, op=OP.mult)
            if nt == 1:
                nc.scalar.activation(lnout_tiles[sg % 2][:], et[:], AF.Ln,
                                     bias=1.0, scale=1.0,
                                     accum_out=acc_sp[:, sg:sg + 1])
                nc.vector.reduce_sum(acc_xt[:, sg:sg + 1], xts[:],
                                     axis=mybir.AxisListType.X)

        # interleaved emission: stage B / BCE of subgroup sg rides inside
        # subgroup sg+1's conv1 stream
        pending = []   # (sg, nt) stage-B chunks not yet emitted
        for a in range(APC):
            emit_front(a)
            if pending:
                emit_back(*pending.pop(0))
            if a % GRP == GRP - 1:
                pending += [(a // GRP, 0), (a // GRP, 1)]
        for args in pending:
            emit_back(*args)

        # ---- final: out[j] = sum_sg (sp - xt) ----
        diff = consts.tile([GRP, NSG], f32)
        nc.vector.tensor_tensor(out=diff[:], in0=acc_sp[:], in1=acc_xt[:],
                                op=OP.subtract)
        osb = consts.tile([GRP, 1], f32)
        nc.vector.reduce_sum(osb[:], diff[:], axis=mybir.AxisListType.X)
        nc.sync.dma_start(out=outp[0:GRP, 0:1], in_=osb[:])

    nc.compile()
    return nc


def _get_program():
    if "nc" not in _cache:
        _cache["nc"] = _build_program()
    return _cache["nc"]


def host_inputs(feature_map, seg, anchors, labels, base_classes, W1, b1, W2,
                b2):
    """Device-layout input maps for all cores (shared tensors prepared once)."""
    import ml_dtypes
    f8 = ml_dtypes.float8_e4m3

    feature_map = np.ascontiguousarray(feature_map, dtype=np.float32)
    seg = np.ascontiguousarray(seg, dtype=np.int32)
    anchors = np.asarray(anchors, dtype=np.int32)
    labels = np.asarray(labels, dtype=np.int32)
    base_classes = np.asarray(base_classes, dtype=np.int32)
    W1 = np.asarray(W1, dtype=np.float32)
    b1 = np.asarray(b1, dtype=np.float32)
    W2 = np.asarray(W2, dtype=np.float32)
    b2 = np.asarray(b2, dtype=np.float32)

    feat8 = np.ascontiguousarray(feature_map.astype(f8))

    # w1: [ci, pair(6) x half(2) x tile(2) x co(128)], pairs (t,t+6)/(t,0)
    w1h = np.zeros((C, 6, 2, 2, 128), dtype=np.float32)
    for p, (tA, tB) in enumerate(C1PAIRS):
        for hf in range(2):
            w1h[:, p, hf, 0, :] = (
                W1[hf * 128:(hf + 1) * 128, :, tA // 3, tA % 3].T * W1S)
            if tB is not None:
                w1h[:, p, hf, 1, :] = (
                    W1[hf * 128:(hf + 1) * 128, :, tB // 3, tB % 3].T * W1S)
    w1h = np.ascontiguousarray(w1h.reshape(C, 3072).astype(f8))

    # w2: [ci, tile(2 halves) x 16 (9 taps + pad)]
    w2h = np.zeros((C, 2, 16), dtype=np.float32)
    for hf in range(2):
        w2h[:, hf, 0:9] = W2[0, hf * C:(hf + 1) * C].reshape(C, 9) * W2S
    w2h = np.ascontiguousarray(w2h.reshape(C, 32).astype(f8))

    # e9 pairs: [part, pair(6) x tile(2) x 16 (4 anchors + pad)]
    e9 = np.zeros((C, 6, 2, 16), dtype=np.float32)
    for p, (tA, tB) in enumerate(C1PAIRS):
        for j in range(GRP):
            e9[32 * j + tA, p, 0, j] = 1.0
            if tB is not None:
                e9[32 * j + tB, p, 1, j] = 1.0
    e9 = np.ascontiguousarray(e9.reshape(C, 192).astype(f8))

    b1h = np.ascontiguousarray(b1.reshape(2, C).T * W1S)
    b2h = np.zeros((C, 1), dtype=np.float32)
    for j in range(GRP):
        b2h[32 * j + 4, 0] = b2[0]

    tgt_cls = base_classes[labels].astype(np.float32)
    y0 = anchors[:, 2].astype(np.int32)
    x0 = anchors[:, 0].astype(np.int32)

    in_maps = []
    for c in range(NCORES):
        sl = slice(c * APC, (c + 1) * APC)
        coords = np.concatenate([y0[sl], x0[sl]]).reshape(1, 2 * APC)
        clsv = np.ascontiguousarray(tgt_cls[sl].reshape(NSG, GRP).T)
        in_maps.append({
            "feat": feat8,
            "seg": seg,
            "coords": np.ascontiguousarray(coords, dtype=np.int32),
            "clsv": clsv,
            "w1t": w1h,
            "w2t": w2h,
            "e9t": e9,
            "b1t": b1h,
            "b2t": b2h,
        })
    return in_maps


def kernel(feature_map, seg, anchors, labels, base_classes, W1, b1, W2, b2):
    global last_exec_time_ns, last_results
    import os
    from concourse.bass_utils import run_bass_kernel_spmd

    in_maps = host_inputs(feature_map, seg, anchors, labels, base_classes,
                          W1, b1, W2, b2)
    nc = _get_program()

    trace = os.environ.get("BASS_KERNEL_TRACE", "0") == "1"
    try:
        rb = run_bass_kernel_spmd(nc, in_maps, list(range(NCORES)),
                                  trace=trace)
    except ModuleNotFoundError:
        rb = run_bass_kernel_spmd(nc, in_maps, list(range(NCORES)),
                                  trace=False)
    last_results = rb
    last_exec_time_ns = rb.exec_time_ns

    partials = [float(rb.results[c]["out"].sum(dtype=np.float64))
                for c in range(NCORES)]
    total = sum(partials) / CROP / CROP / (NANCH + 1e-10)
    return np.float32(total)


# revision 15
# speedup vs baseline: 2.5661x; 1.3907x over previous
"""Trainium2 Bass kernel for BinaryMaskPredictor (ragged anchors), fp8 rev 2.

Data-parallel over the 256 anchors: 32 anchors per NeuronCore on 8 cores.
feature_map / seg / conv weights are replicated; per-core anchor coords and
target classes are sharded.  Each core computes sum over its anchors of
sum_px BCE(logits, tgt); the host sums the 8x4 partials and normalizes.

Structure (all matmuls fp8e4; DoubleRow = 0.5 cyc/row):
  * Crops live in a DENSE pitch-32 domain (no pad columns).  Walrus
    requires DoubleRow K-tiles 64 bytes apart, so taps pair as
    (dy, dx)+(dy+2, dx): three real pairs (t0,t6),(t1,t7),(t2,t8) plus
    three zero-padded singles (t3),(t4),(t5).  Vertical padding is two
    zero rows above/below the crop.  Horizontal SAME-padding is fixed up
    afterwards: a dx=0 tap wrongly reads the previous row's col 31 at
    out col 0 (and dx=2 the next row's col 0 at out col 31); per tap a
    tiny negated-weight matmul on the same addresses cancels the bogus
    contribution exactly.  1024 true outputs per half = 2 PSUM banks.
  * h is dense [128, 2*1024] fp8 -> stage A (Z = W2^T h) is one DoubleRow
    matmul per 512-chunk contracting both ci halves (half stride 1024).
  * Z copies flat [9,1024] into a dense pitch-32 z tile; stage B uses the
    same pair+fixup scheme with unit/negated-unit e-columns (K=105,
    anchors stacked at partition 32j).
  * BCE: sum(bce) = sum(ln(1+exp(L))) - sum(L*tgt); exp/ln on ACT with
    accumulate, tgt compare on GPSIMD (int32 seg vs f32 class id),
    mult + row-reduce on DVE.  b2 rides the center-tap row of the Z copy
    bias; b1 rides the conv1 relu bias; fp8 weight scaling (W1*32,
    W2*64) is undone by the Z-copy scale 1/2048.
  * stage B / BCE of subgroup s are emitted interleaved into subgroup
    s+1's conv1 stream so the 2-bank lt tile never stalls PE.
PSUM: c1 2x[128,1024] (4 banks) + zps [16,1024] (2) + lt [4,1024] (2) = 8.
"""

import numpy as np
from contextlib import ExitStack

C = 128
HF = WF = 320
IMG = 1280
NANCH = 256
CROP = 32
XD = 64                   # x/z tile offset of crop pixel (0,0) (2 pad rows)
XLEN = 1160               # 64 + 32*34 + tail slack
NCORES = 8
APC = NANCH // NCORES     # 32 anchors per core
GRP = 4                   # anchors per stage-B subgroup
NSG = APC // GRP          # 8 subgroups per core
W1S = 32.0                # fp8 scale for W1 (+b1)
W2S = 64.0                # fp8 scale for W2
ZSC = 1.0 / (W1S * W2S)   # undo scaling at the Z copy
# DoubleRow pairs (tap t=(dy,dx)=(t//3,t%3)): (t, t+6) at stride 64
C1PAIRS = [(0, 6), (1, 7), (2, 8), (3, None), (4, None), (5, None)]
FIXTAPS = [0, 3, 6, 2, 5, 8]   # dx=0 taps fix col 0; dx=2 taps fix col 31

_cache = {}
last_exec_time_ns = None
last_results = None


def _build_program():
    import concourse.bass as bass
    import concourse.tile as tile
    import concourse.mybir as mybir
    from concourse import bacc
    from concourse.bass import ds

    f32 = mybir.dt.float32
    f8 = mybir.dt.float8e4
    i32 = mybir.dt.int32
    AF = mybir.ActivationFunctionType
    OP = mybir.AluOpType
    PM = mybir.MatmulPerfMode

    nc = bacc.Bacc("TRN2", target_bir_lowering=False, debug=False,
                   num_devices=NCORES)

    feat = nc.declare_dram_parameter("feat", [C, HF, WF], f8, isOutput=False)
    seg = nc.declare_dram_parameter("seg", [IMG, IMG], i32, isOutput=False)
    coords = nc.declare_dram_parameter("coords", [1, 2 * APC], i32,
                                       isOutput=False)
    clsv = nc.declare_dram_parameter("clsv", [GRP, NSG], f32, isOutput=False)
    w1t = nc.declare_dram_parameter("w1t", [C, 3072], f8, isOutput=False)
    w2t = nc.declare_dram_parameter("w2t", [C, 32], f8, isOutput=False)
    e9t = nc.declare_dram_parameter("e9t", [C, 192], f8, isOutput=False)
    b1t = nc.declare_dram_parameter("b1t", [C, 2], f32, isOutput=False)
    b2t = nc.declare_dram_parameter("b2t", [C, 1], f32, isOutput=False)
    outp = nc.declare_dram_parameter("out", [GRP, 1], f32, isOutput=True)

    seg4 = seg[:].rearrange("(h a) (w b) -> h a w b", a=4, b=4)

    SP_ONLY = (mybir.EngineType.SP,)
    POOL_ONLY = (mybir.EngineType.Pool,)

    XR, HR, ZR, MR = 6, 4, 2, 2

    def view(t, off, dims, nparts=None):
        v = t[:]
        p0 = list(v.ap[0]) if nparts is None else [v.ap[0][0], nparts]
        return bass.AP(v.tensor, v.offset + off, [p0] + dims)

    with ExitStack() as ctx:
        tc = ctx.enter_context(tile.TileContext(nc))

        consts = ctx.enter_context(tc.tile_pool(name="consts", bufs=1))

        c1p = ctx.enter_context(tc.tile_pool(name="c1p", bufs=2, space="PSUM"))
        zpp = ctx.enter_context(tc.tile_pool(name="zpp", bufs=1, space="PSUM"))
        ltp = ctx.enter_context(tc.tile_pool(name="ltp", bufs=1, space="PSUM"))
        sdp = ctx.enter_context(tc.tile_pool(name="sdp", bufs=1, space="PSUM"))

        # ---- weights / constants ----
        w1_sb = consts.tile([C, 3072], f8)
        nc.sync.dma_start(out=w1_sb[:, 0:512], in_=w1t[:, 0:512])
        nc.sync.dma_start(out=w1_sb[:, 512:3072], in_=w1t[:, 512:3072])
        w2_sb = consts.tile([C, 32], f8)
        nc.sync.dma_start(out=w2_sb[:], in_=w2t[:])
        e9_sb = consts.tile([C, 192], f8)
        nc.sync.dma_start(out=e9_sb[:], in_=e9t[:])
        b1_sb = consts.tile([C, 2], f32)
        nc.sync.dma_start(out=b1_sb[:], in_=b1t[:])
        b2_sb = consts.tile([C, 1], f32)
        nc.sync.dma_start(out=b2_sb[:], in_=b2t[:])
        cls_sb = consts.tile([GRP, NSG], f32)
        nc.sync.dma_start(out=cls_sb[:], in_=clsv[:])
        coords_sb = consts.tile([1, 2 * APC], i32)
        nc.sync.dma_start(out=coords_sb[:], in_=coords[:])

        # persistent tiles; pad rows zeroed once (DMA / copies only ever
        # write the crop interior, so the zero pads survive reuse)
        x_tiles = [consts.tile([C, XLEN], f8, name=f"xt{i}")
                   for i in range(XR)]
        z_tiles = [consts.tile([C, XLEN], f8, name=f"zt{i}")
                   for i in range(ZR)]
        for t in x_tiles:
            nc.any.memset(t[:, 0:XD], 0.0)
            nc.any.memset(t[:, XD + 1024:XLEN], 0.0)
        for t in z_tiles:
            nc.any.memset(t[:], 0.0)
        h_tiles = [consts.tile([C, 2048], f8, name=f"ht{i}")
                   for i in range(HR)]
        mseg_tiles = [consts.tile([GRP, 1024], i32, name=f"mt{i}")
                      for i in range(MR)]
        tgt_tiles = [consts.tile([GRP, 1024], f32, name=f"tt{i}")
                     for i in range(2)]
        e_tiles = [consts.tile([GRP, 1024], f32, name=f"et{i}")
                   for i in range(2)]
        xts_tiles = [consts.tile([GRP, 1024], f32, name=f"xx{i}")
                     for i in range(2)]
        lnout_tiles = [consts.tile([GRP, 1024], f32, name=f"lo{i}")
                       for i in range(2)]
        acc_sp = consts.tile([GRP, NSG], f32)
        acc_xt = consts.tile([GRP, NSG], f32)

        zps = zpp.tile([16, 1024], f32)    # stage-A out, ring 1 (2 banks)
        ltt = ltp.tile([GRP, 512], f32)    # stage-B out chunk, ring 1 (1 bank)
        side = sdp.tile([C, 192], f32)     # edge-column psum, ring 1 (1 bank)
        # single-tap lhsT offsets in the pair layouts: tap -> (pair, tile)
        def tap_pt(t):
            if t < 3:
                return t, 0
            if t >= 6:
                return t - 6, 1
            return t, 0

        def emit_front(a):
            """conv1 + relu + stage A + z copy for anchor a."""
            sg, j = divmod(a, GRP)
            xt = x_tiles[a % XR]
            ht = h_tiles[a % HR]
            z_sb = z_tiles[sg % ZR]
            mseg_t = mseg_tiles[sg % MR]

            # dynamic crop DMAs
            yv = nc.values_load(coords_sb[0:1, a:a + 1], engines=SP_ONLY,
                                min_val=0, max_val=HF - CROP,
                                skip_runtime_bounds_check=True)
            xv = nc.values_load(coords_sb[0:1, APC + a:APC + a + 1],
                                engines=SP_ONLY, min_val=0,
                                max_val=WF - CROP,
                                skip_runtime_bounds_check=True)
            nc.sync.dma_start(
                out=view(xt, XD, [[CROP, CROP], [1, CROP]]),
                in_=feat[:, ds(yv, CROP), ds(xv, CROP)],
            )
            for v in (yv, xv):
                for reg in v.val.handles:
                    nc.free_register(reg)

            ya = nc.values_load(coords_sb[0:1, a:a + 1], engines=POOL_ONLY,
                                min_val=0, max_val=HF - CROP,
                                skip_runtime_bounds_check=True)
            xa = nc.values_load(coords_sb[0:1, APC + a:APC + a + 1],
                                engines=POOL_ONLY, min_val=0,
                                max_val=WF - CROP,
                                skip_runtime_bounds_check=True)
            nc.gpsimd.dma_start(
                out=mseg_t[j:j + 1, 0:1024],
                in_=seg4[ds(ya, CROP), 0, ds(xa, CROP), 0],
            )
            for v in (ya, xa):
                for reg in v.val.handles:
                    nc.free_register(reg)

            # conv1: dense out f = 32*y + x; tap (dy,dx) reads the x tile
            # at f + 32*dy + dx + XD - 33; pairs (t, t+6) sit 64 apart
            for hf in range(2):
                ct = c1p.tile([C, 1024], f32, tag="c1", name=f"c1_{a}_{hf}")
                for ci in range(2):
                    coff = 512 * ci
                    for p, (tA, tB) in enumerate(C1PAIRS):
                        dyA, dxA = tA // 3, tA % 3
                        offA = XD - 33 + coff + 32 * dyA + dxA
                        lhs = view(w1_sb, p * 512 + hf * 256,
                                   [[128, 2], [1, 128]])
                        rhs = view(xt, offA, [[64, 2], [1, 512]])
                        nc.tensor.matmul(ct[:, coff:coff + 512], lhs, rhs,
                                         start=(p == 0), stop=(p == 5),
                                         perf_mode=PM.DoubleRow)
                # edge columns 0/31 computed clean in the side tile
                # (main-pass values there read across rows; overwritten in h)
                for ei, ocol in enumerate((0, 31)):
                    sbase = hf * 64 + ei * 32
                    taps = [t for t in range(9)
                            if (t % 3 >= 1 if ocol == 0 else t % 3 <= 1)]
                    for ti, t in enumerate(taps):
                        dy, dx = t // 3, t % 3
                        rhso = XD + 32 * (dy - 1) + (dx - 1 + ocol)
                        p, tl = tap_pt(t)
                        lhs = view(w1_sb, p * 512 + hf * 256 + tl * 128,
                                   [[1, 128]])
                        rhs = view(xt, rhso, [[32, CROP]])
                        nc.tensor.matmul(side[:, sbase:sbase + 32], lhs, rhs,
                                         start=(ti == 0),
                                         stop=(ti == len(taps) - 1),
                                         skip_group_check=True)
                # bias + relu -> dense fp8 h (edge cols overwritten after)
                hview = ht[:, hf * 1024:hf * 1024 + 1024]
                if hf == (a % 2):
                    nc.scalar.activation(hview, ct[:], AF.Relu,
                                         bias=b1_sb[:, hf:hf + 1], scale=1.0)
                else:
                    nc.vector.tensor_scalar(
                        out=hview, in0=ct[:],
                        scalar1=b1_sb[:, hf:hf + 1], scalar2=0.0,
                        op0=OP.add, op1=OP.max)
                for ei, ocol in enumerate((0, 31)):
                    sbase = hf * 64 + ei * 32
                    hcol = view(ht, hf * 1024 + ocol, [[CROP, CROP]])
                    if hf == (a % 2):
                        nc.scalar.activation(hcol, side[:, sbase:sbase + 32],
                                             AF.Relu,
                                             bias=b1_sb[:, hf:hf + 1],
                                             scale=1.0)
                    else:
                        nc.vector.tensor_scalar(
                            out=hcol, in0=side[:, sbase:sbase + 32],
                            scalar1=b1_sb[:, hf:hf + 1], scalar2=0.0,
                            op0=OP.add, op1=OP.max)

            # stage A: Z[9, f] over dense h (DR over ci halves, stride 1024)
            for ci in range(2):
                coff = 512 * ci
                rhs = view(ht, coff, [[1024, 2], [1, 512]])
                lhs = view(w2_sb, 0, [[16, 2], [1, 9]])
                nc.tensor.matmul(zps[0:9, coff:coff + 512], lhs, rhs,
                                 start=True, stop=True,
                                 perf_mode=PM.DoubleRow)

            # Z copy: flat [9,1024], *1/2048, +b2 on center-tap row
            zout = view(z_sb, XD + 32 * j * XLEN, [[1, 1024]], 9)
            if a % 2 == 0:
                nc.scalar.activation(zout, zps[0:9, 0:1024], AF.Identity,
                                     bias=b2_sb[32 * j:32 * j + 9, 0:1],
                                     scale=ZSC)
            else:
                nc.vector.tensor_scalar(
                    out=zout, in0=zps[0:9, 0:1024], scalar1=ZSC,
                    scalar2=b2_sb[32 * j:32 * j + 9, 0:1],
                    op0=OP.mult, op1=OP.add)

        def emit_back(sg, nt):
            """stage B chunk nt (512 outs) + its BCE for subgroup sg."""
            z_sb = z_tiles[sg % ZR]
            mseg_t = mseg_tiles[sg % MR]
            tgt = tgt_tiles[sg % 2]
            et = e_tiles[sg % 2]
            xts = xts_tiles[sg % 2]

            if nt == 0:
                nc.gpsimd.tensor_scalar(
                    out=tgt[:], in0=mseg_t[:],
                    scalar1=cls_sb[0:GRP, sg:sg + 1], scalar2=None,
                    op0=OP.is_equal)

            coff = 512 * nt
            for p, (tA, tB) in enumerate(C1PAIRS):
                dyA, dxA = tA // 3, tA % 3
                offA = XD - 33 + coff + 32 * dyA + dxA
                lhs = view(e9_sb, p * 32, [[16, 2], [1, 4]], 105)
                rhs = view(z_sb, offA, [[64, 2], [1, 512]], 105)
                nc.tensor.matmul(ltt[:], lhs, rhs,
                                 start=(p == 0), stop=(p == 5),
                                 perf_mode=PM.DoubleRow)

            if nt == 0:
                # clean edge columns of the logits into the side tile
                for ei, ocol in enumerate((0, 31)):
                    sbase = 128 + ei * 32
                    taps = [t for t in range(9)
                            if (t % 3 >= 1 if ocol == 0 else t % 3 <= 1)]
                    for ti, t in enumerate(taps):
                        dy, dx = t // 3, t % 3
                        rhso = XD + 32 * (dy - 1) + (dx - 1 + ocol)
                        p, tl = tap_pt(t)
                        lhs = view(e9_sb, p * 32 + tl * 16, [[1, 4]], 105)
                        rhs = view(z_sb, rhso, [[32, CROP]], 105)
                        nc.tensor.matmul(side[0:GRP, sbase:sbase + 32],
                                         lhs, rhs,
                                         start=(ti == 0),
                                         stop=(ti == len(taps) - 1),
                                         skip_group_check=True)
                for ei, ocol in enumerate((0, 31)):
                    sbase = 128 + ei * 32
                    ecol = view(et, ocol, [[CROP, CROP]])
                    nc.scalar.activation(ecol, side[0:GRP, sbase:sbase + 32],
                                         AF.Exp, bias=0.0, scale=1.0)
                    xcol = view(xts, ocol, [[CROP, CROP]])
                    tcol = view(tgt, ocol, [[CROP, CROP]])
                    nc.vector.tensor_tensor(
                        out=xcol, in0=side[0:GRP, sbase:sbase + 32],
                        in1=tcol, op=OP.mult)

            # interior columns 1..30 from the chunk psum
            lint = view(ltt, 1, [[CROP, 16], [1, 30]])
            eint = view(et, coff + 1, [[CROP, 16], [1, 30]])
            nc.scalar.activation(eint, lint, AF.Exp, bias=0.0, scale=1.0)
            xint = view(xts, coff + 1, [[CROP, 16], [1, 30]])
            tint = view(tgt, coff + 1, [[CROP, 16], [1, 30]])
            nc.vector.tensor_tensor(out=xint, in0=lint, in1=tint, op=OP.mult)
            if nt == 1:
                nc.scalar.activation(lnout_tiles[sg % 2][:], et[:], AF.Ln,
                                     bias=1.0, scale=1.0,
                                     accum_out=acc_sp[:, sg:sg + 1])
                nc.vector.reduce_sum(acc_xt[:, sg:sg + 1], xts[:],
                                     axis=mybir.AxisListType.X)

        # interleaved emission: stage B / BCE of subgroup sg rides inside
        # subgroup sg+1's conv1 stream
        pending = []   # (sg, nt) stage-B chunks not yet emitted
        for a in range(APC):
            emit_front(a)
            if pending:
                emit_back(*pending.pop(0))
            if a % GRP == GRP - 1:
                pending += [(a // GRP, 0), (a // GRP, 1)]
        for args in pending:
            emit_back(*args)

        # ---- final: out[j] = sum_sg (sp - xt) ----
        diff = consts.tile([GRP, NSG], f32)
        nc.vector.tensor_tensor(out=diff[:], in0=acc_sp[:], in1=acc_xt[:],
                                op=OP.subtract)
        osb = consts.tile([GRP, 1], f32)
        nc.vector.reduce_sum(osb[:], diff[:], axis=mybir.AxisListType.X)
        nc.sync.dma_start(out=outp[0:GRP, 0:1], in_=osb[:])

    nc.compile()
    return nc


def _get_program():
    if "nc" not in _cache:
        _cache["nc"] = _build_program()
    return _cache["nc"]


def host_inputs(feature_map, seg, anchors, labels, base_classes, W1, b1, W2,
                b2):
    """Device-layout input maps for all cores (shared tensors prepared once)."""
    import ml_dtypes
    f8 = ml_dtypes.float8_e4m3

    feature_map = np.ascontiguousarray(feature_map, dtype=np.float32)
    seg = np.ascontiguousarray(seg, dtype=np.int32)
    anchors = np.asarray(anchors, dtype=np.int32)
    labels = np.asarray(labels, dtype=np.int32)
    base_classes = np.asarray(base_classes, dtype=np.int32)
    W1 = np.asarray(W1, dtype=np.float32)
    b1 = np.asarray(b1, dtype=np.float32)
    W2 = np.asarray(W2, dtype=np.float32)
    b2 = np.asarray(b2, dtype=np.float32)

    feat8 = np.ascontiguousarray(feature_map.astype(f8))

    # w1: [ci, pair(6) x half(2) x tile(2) x co(128)], pairs (t,t+6)/(t,0)
    w1h = np.zeros((C, 6, 2, 2, 128), dtype=np.float32)
    for p, (tA, tB) in enumerate(C1PAIRS):
        for hf in range(2):
            w1h[:, p, hf, 0, :] = (
                W1[hf * 128:(hf + 1) * 128, :, tA // 3, tA % 3].T * W1S)
            if tB is not None:
                w1h[:, p, hf, 1, :] = (
                    W1[hf * 128:(hf + 1) * 128, :, tB // 3, tB % 3].T * W1S)
    w1h = np.ascontiguousarray(w1h.reshape(C, 3072).astype(f8))

    # w2: [ci, tile(2 halves) x 16 (9 taps + pad)]
    w2h = np.zeros((C, 2, 16), dtype=np.float32)
    for hf in range(2):
        w2h[:, hf, 0:9] = W2[0, hf * C:(hf + 1) * C].reshape(C, 9) * W2S
    w2h = np.ascontiguousarray(w2h.reshape(C, 32).astype(f8))

    # e9 pairs: [part, pair(6) x tile(2) x 16 (4 anchors + pad)]
    e9 = np.zeros((C, 6, 2, 16), dtype=np.float32)
    for p, (tA, tB) in enumerate(C1PAIRS):
        for j in range(GRP):
            e9[32 * j + tA, p, 0, j] = 1.0
            if tB is not None:
                e9[32 * j + tB, p, 1, j] = 1.0
    e9 = np.ascontiguousarray(e9.reshape(C, 192).astype(f8))

    b1h = np.ascontiguousarray(b1.reshape(2, C).T * W1S)
    b2h = np.zeros((C, 1), dtype=np.float32)
    for j in range(GRP):
        b2h[32 * j + 4, 0] = b2[0]

    tgt_cls = base_classes[labels].astype(np.float32)
    y0 = anchors[:, 2].astype(np.int32)
    x0 = anchors[:, 0].astype(np.int32)

    in_maps = []
    for c in range(NCORES):
        sl = slice(c * APC, (c + 1) * APC)
        coords = np.concatenate([y0[sl], x0[sl]]).reshape(1, 2 * APC)
        clsv = np.ascontiguousarray(tgt_cls[sl].reshape(NSG, GRP).T)
        in_maps.append({
            "feat": feat8,
            "seg": seg,
            "coords": np.ascontiguousarray(coords, dtype=np.int32),
            "clsv": clsv,
            "w1t": w1h,
            "w2t": w2h,
            "e9t": e9,
            "b1t": b1h,
            "b2t": b2h,
        })
    return in_maps


def kernel(feature_map, seg, anchors, labels, base_classes, W1, b1, W2, b2):
    global last_exec_time_ns, last_results
    import os
    from concourse.bass_utils import run_bass_kernel_spmd

    in_maps = host_inputs(feature_map, seg, anchors, labels, base_classes,
                          W1, b1, W2, b2)
    nc = _get_program()

    trace = os.environ.get("BASS_KERNEL_TRACE", "0") == "1"
    try:
        rb = run_bass_kernel_spmd(nc, in_maps, list(range(NCORES)),
                                  trace=trace)
    except ModuleNotFoundError:
        rb = run_bass_kernel_spmd(nc, in_maps, list(range(NCORES)),
                                  trace=False)
    last_results = rb
    last_exec_time_ns = rb.exec_time_ns

    partials = [float(rb.results[c]["out"].sum(dtype=np.float64))
                for c in range(NCORES)]
    total = sum(partials) / CROP / CROP / (NANCH + 1e-10)
    return np.float32(total)
